# revision 4
# baseline (speedup 1.0000x reference)
"""Darknet 3x3 conv block (conv * mask + bias) via Winograd F(2x2,3x3) on 8 TRN2 cores.

Problem: x[1,512,192,192] (*) w[512,512,3,3] stride1 pad1, then *mask + bias.

Strategy (per core, H-sharded: 24 output rows = 12x96 2x2-tiles):
  - Host: pad x and pack col-parity planes [128c_l, cc4, 26, 4, 98] bf16
    (planes: even cols, even+2, odd, odd+2 - makes every device-side
    column combo a stride-1 aligned op, 2x DVE mode); weights
    U = G w G^T packed [128c_l, fm4, cc4, uv16, 128f] bf16; mask
    [128, ch4, p2, q2, 288] bf16; bias [128, fm4] f32.
  - Device, 4 chunks of 3 tile-rows (288 tiles):
    * DVE input transform: col combos on parity planes (2x), then row
      combos (2x): V[cc,u,vl,tile].
    * PE: for v fixed, psum[4u,288] accumulates 16 MMs (u4 x cc4) of
      lhsT=U[c,f], rhs=V[c,288]; 1024 MMs of width 288 total.
    * Act evicts psum -> mc bf16; DVE t0 = m0+m1+m2 (in-place chain),
      GpSimd t1 = m1-m2-m3; DVE col stage y0 = t0+t1+t2, y1 = t1-t2-t3
      + mask-mul; Act bias-add; bf16 DMA out (host casts fp32).
  - DVE queue interleaving: next chunk's input transforms are emitted
    into the eviction-wait gaps so the DVE never idles behind the PE.
"""

import sys

for _p in ("/opt/trn_rl_repo",):
    if _p not in sys.path:
        sys.path.insert(0, _p)

import numpy as np
import ml_dtypes

N_CORES = 8
C = 512
F = 512
H = 192
W = 192
HC = H // N_CORES          # output rows per core = 24
CC = C // 128
FM = F // 128
NCH = 4                    # chunks per core
TRC = 3                    # tile-rows per chunk
TW = 96                    # tile-cols
PX = TRC * TW              # tiles per chunk = 288
XR = 2 * TRC + 2           # x rows per chunk = 8
NWARM = 8

_CACHE = {}


def _build():
    import concourse.bacc as bacc
    import concourse.mybir as mybir
    from concourse.tile import TileContext

    BF = mybir.dt.bfloat16
    F32 = mybir.dt.float32
    IDENT = mybir.ActivationFunctionType.Identity

    nc = bacc.Bacc(trn_type="TRN2", num_devices=N_CORES)
    x_sh = nc.dram_tensor("x_sh", [128, 2, HC + 2, CC, 2, 98], BF,
                          kind="ExternalInput")
    u_sh = nc.dram_tensor("u_sh", [128, FM, CC, 16, 128], BF,
                          kind="ExternalInput")
    m_sh = nc.dram_tensor("m_sh", [128, NCH, 2, 2, PX], BF,
                          kind="ExternalInput")
    b_sh = nc.dram_tensor("b_sh", [128, FM], F32, kind="ExternalInput")
    y_sh = nc.dram_tensor("y_sh", [FM, 128, NCH, 2, 2, PX], BF,
                          kind="ExternalOutput")

    with TileContext(nc) as tc:
        with (
            tc.tile_pool(name="const", bufs=1) as cpool,
            tc.tile_pool(name="xin", bufs=2) as xpool,
            tc.tile_pool(name="ein", bufs=1) as epool,
            tc.tile_pool(name="vin", bufs=4) as vpool,
            tc.tile_pool(name="psum", bufs=2, space="PSUM") as ppool,
            tc.tile_pool(name="mcp", bufs=3) as mcpool,
            tc.tile_pool(name="tp", bufs=4) as tpool,
            tc.tile_pool(name="mtp", bufs=2) as mtpool,
            tc.tile_pool(name="ymp", bufs=2) as ympool,
            tc.tile_pool(name="y2p", bufs=2) as y2pool,
        ):
            # PE warmup while first DMAs land
            scratch = cpool.tile([128, PX], BF)
            nc.vector.memset(scratch[:], 0.0)
            dps = ppool.tile([128, 4, 512], F32, name="dps", tag="ps")
            for _ in range(NWARM):
                nc.tensor.matmul(dps[:, 0, :PX], scratch[:, :128], scratch[:],
                                 start=True, stop=True)

            ut = cpool.tile([128, FM, CC, 16, 128], BF)
            bt = cpool.tile([128, FM], F32)
            nc.scalar.dma_start(out=ut[:, 0], in_=u_sh[:, 0])
            nc.scalar.dma_start(out=bt[:], in_=b_sh[:])
            for fm in range(1, FM):
                nc.scalar.dma_start(out=ut[:, fm], in_=u_sh[:, fm])

            xts = [None] * NCH

            def dma_x(ch):
                # split by plane-pair so the first col-combo can start as
                # soon as planes {0,1} land; rows-major with cc inner ->
                # 12.5KB contiguous blocks per descriptor
                xt = xpool.tile([128, 2, XR, CC, 2, 98], BF, name=f"x_{ch}",
                                tag="x")
                r0 = 2 * TRC * ch
                nc.sync.dma_start(out=xt[:, 0],
                                  in_=x_sh[:, 0, r0:r0 + XR])
                # pair1 rides the Act ring once the weights are through it
                eng = nc.sync if ch == 0 else nc.scalar
                eng.dma_start(out=xt[:, 1], in_=x_sh[:, 1, r0:r0 + XR])
                return xt

            def in12(ch, v):
                # col combo on parity planes (2x) then row combos (2x)
                xt = xts[ch]
                p0 = xt[:, 0, :, :, 0, :TW]
                p1 = xt[:, 0, :, :, 1, :TW]
                p2 = xt[:, 1, :, :, 0, :TW]
                p3 = xt[:, 1, :, :, 1, :TW]
                combo = {0: ('sub', p0, p1), 1: ('add', p2, p1),
                         2: ('sub', p1, p2), 3: ('sub', p2, p3)}
                vt = vpool.tile([128, 4, TRC, CC, TW], BF,
                                name=f"v_{ch}_{v}", tag="v")
                et = epool.tile([128, XR, CC, TW], BF, name=f"e_{v}", tag="e")
                op, a, bb = combo[v]
                getattr(nc.vector, f"tensor_{op}")(et[:], a, bb)
                r0 = et[:, 0:2 * TRC - 1:2, :, :]
                r1 = et[:, 1:2 * TRC:2, :, :]
                r2 = et[:, 2:2 * TRC + 1:2, :, :]
                r3 = et[:, 3:2 * TRC + 2:2, :, :]
                nc.vector.tensor_sub(vt[:, 0], r0, r2)
                nc.vector.tensor_add(vt[:, 1], r1, r2)
                nc.vector.tensor_sub(vt[:, 2], r2, r1)
                nc.vector.tensor_sub(vt[:, 3], r1, r3)
                return vt

            xts[0] = dma_x(0)
            vts = {}
            for v in range(2):
                vts[(0, v)] = in12(0, v)

            for ch in range(NCH):
                if ch + 1 < NCH:
                    xts[ch + 1] = dma_x(ch + 1)
                mt = mtpool.tile([128, 2, 2, PX], BF, name=f"m_{ch}", tag="m")
                nc.scalar.dma_start(out=mt[:], in_=m_sh[:, ch])

                tts = [tpool.tile([128, 2, 4, PX], BF, name=f"t_{ch}_{fm}",
                                  tag="t") for fm in range(FM)]

                for h in range(2):
                    vth = [vts.pop((ch, 2 * h)), vts.pop((ch, 2 * h + 1))]
                    for fm in range(FM):
                        mch = mcpool.tile([128, 4, 2, PX], BF,
                                          name=f"mc_{ch}_{h}_{fm}", tag="mc")
                        for vl in range(2):
                            v = 2 * h + vl
                            pt = ppool.tile([128, 4, 512], F32,
                                            name=f"ps_{ch}_{h}_{fm}_{vl}",
                                            tag="ps")
                            for u in range(4):
                                for cc in range(CC):
                                    nc.tensor.matmul(
                                        pt[:, u, :PX],
                                        ut[:, fm, cc, u * 4 + v],
                                        vth[vl][:, u, :, cc, :],
                                        start=(cc == 0), stop=(cc == CC - 1),
                                    )
                            nc.scalar.activation(mch[:, :, vl], pt[:, :, :PX],
                                                 IDENT)
                        # out row-transform: t0 on DVE, t1 on GpSimd
                        tt = tts[fm]
                        t0 = tt[:, 0, 2 * h:2 * h + 2]
                        nc.vector.tensor_add(t0, mch[:, 0], mch[:, 1])
                        nc.vector.tensor_add(t0, t0, mch[:, 2])
                        t1 = tt[:, 1, 2 * h:2 * h + 2]
                        nc.gpsimd.tensor_sub(t1, mch[:, 1], mch[:, 2])
                        nc.gpsimd.tensor_sub(t1, t1, mch[:, 3])

                        if h == 1:
                            # out col-transform + mask (DVE), bias (Act), DMA
                            ym = ympool.tile([128, 2, 2, PX], BF,
                                             name=f"ym_{fm}", tag="ym")
                            nc.vector.tensor_add(ym[:, :, 0], tt[:, :, 0],
                                                 tt[:, :, 1])
                            nc.vector.tensor_add(ym[:, :, 0], ym[:, :, 0],
                                                 tt[:, :, 2])
                            nc.vector.tensor_sub(ym[:, :, 1], tt[:, :, 1],
                                                 tt[:, :, 2])
                            nc.vector.tensor_sub(ym[:, :, 1], ym[:, :, 1],
                                                 tt[:, :, 3])
                            nc.vector.tensor_mul(ym[:], ym[:], mt[:])
                            y2 = y2pool.tile([128, 2, 2, PX], BF,
                                             name=f"y2_{fm}", tag="y2")
                            nc.scalar.activation(y2[:], ym[:], IDENT,
                                                 bias=bt[:, fm:fm + 1])
                            nc.sync.dma_start(out=y_sh[fm, :, ch], in_=y2[:])

                    # fill the DVE eviction-wait gap with the upcoming input
                    # transforms, staggered half a chunk so the vpool buffers
                    # they reuse were released by an already-finished GEMM
                    # pass (not the one still running)
                    if h == 0:
                        for vl in range(2):
                            vts[(ch, 2 + vl)] = in12(ch, 2 + vl)
                    elif ch + 1 < NCH:
                        for vl in range(2):
                            vts[(ch + 1, vl)] = in12(ch + 1, vl)

    nc.compile()
    return nc


def _pack(x, w, b, mask):
    x = np.asarray(x, dtype=np.float32)
    w = np.asarray(w, dtype=np.float32)
    b = np.asarray(b, dtype=np.float32)
    mask = np.asarray(mask)

    xp = np.zeros((C, H + 2, W + 2), dtype=np.float32)
    xp[:, 1:-1, 1:-1] = x[0]
    # col-parity planes: [C, H+2, 4, 98]
    x4 = np.zeros((C, H + 2, 4, 98), dtype=np.float32)
    x4[:, :, 0, :97] = xp[:, :, 0::2]      # x[2tc]
    x4[:, :, 1, :96] = xp[:, :, 2::2]      # x[2tc+2]
    x4[:, :, 2, :97] = xp[:, :, 1::2]      # x[2tc+1]
    x4[:, :, 3, :96] = xp[:, :, 3::2]      # x[2tc+3]
    # pair-major over c: [C, 2pair, H+2, 2plane, 98] (per-core transpose
    # below moves pair outside cc)
    x4 = np.ascontiguousarray(
        x4.reshape(C, H + 2, 2, 2, 98).transpose(0, 2, 1, 3, 4))
    x4 = x4.astype(ml_dtypes.bfloat16)

    # U = G w G^T -> [128c_l, fm, cc, u*4+v, f_l]
    G = np.array([[1, 0, 0], [.5, .5, .5], [.5, -.5, .5], [0, 0, 1]],
                 dtype=np.float32)
    U = np.einsum('ui,fcij,vj->uvfc', G, w, G)          # [4,4,F,C]
    U = U.reshape(4, 4, FM, 128, CC, 128)               # [u,v,fm,fl,cc,cl]
    U = U.transpose(5, 2, 4, 0, 1, 3).reshape(128, FM, CC, 16, 128)
    U = np.ascontiguousarray(U).astype(ml_dtypes.bfloat16)

    b_re = np.ascontiguousarray(b.reshape(FM, 128).T)   # [128, FM]

    mf = mask.astype(ml_dtypes.bfloat16)
    in_maps = []
    for k in range(N_CORES):
        xs = x4[:, :, HC * k:HC * k + HC + 2]
        xs = np.ascontiguousarray(
            xs.reshape(CC, 128, 2, HC + 2, 2, 98).transpose(1, 2, 3, 0, 4, 5))
        mk = mf[HC * k:HC * k + HC]                     # [24, 192]
        mk = mk.reshape(NCH, TRC, 2, TW, 2).transpose(0, 2, 4, 1, 3)
        mk = np.ascontiguousarray(mk.reshape(NCH, 2, 2, PX))
        mk = np.broadcast_to(mk[None], (128, NCH, 2, 2, PX))
        in_maps.append({"x_sh": xs, "u_sh": U,
                        "m_sh": np.ascontiguousarray(mk),
                        "b_sh": b_re})
    return in_maps


def _unpack(results):
    slabs = []
    for k in range(N_CORES):
        ys = results[k]["y_sh"]                          # [4,128,4,2,2,288] bf16
        ys = np.asarray(ys).astype(np.float32)
        ys = ys.reshape(FM, 128, NCH, 2, 2, TRC, TW)     # [fm,fl,ch,p,q,t,tc]
        ys = ys.transpose(0, 1, 2, 5, 3, 6, 4)           # [fm,fl,ch,t,p,tc,q]
        slabs.append(ys.reshape(F, HC, W))
    out = np.concatenate(slabs, axis=1)
    return out[None]


def _run(inputs, **run_kwargs):
    from concourse.bass_utils import run_bass_kernel_spmd

    if "nc" not in _CACHE:
        _CACHE["nc"] = _build()
    nc = _CACHE["nc"]
    in_maps = _pack(inputs["x"], inputs["w"], inputs["b"], inputs["mask"])
    res = run_bass_kernel_spmd(nc, in_maps, core_ids=list(range(N_CORES)), **run_kwargs)
    return _unpack(res.results), res


def kernel(**inputs):
    out, _ = _run(inputs)
    return out
